# revision 1
# baseline (speedup 1.0000x reference)
"""Trainium2 Bass kernel for nn_Dilation2D (101x101 grayscale dilation with a
parabolic structuring element).

Math: out[r, c] = max_{i,j} padded[i + c, j + r] + h[i, j] with
h[i, j] = -(z_i^2 + z_j^2) / (4 s) separable into f(i) + g(j), so the 2D
max-plus convolution factors into two 1D sliding passes:

  stage 1:  t[p, r] = max_j rowpad[p, j + r] + w[j]     (slide along columns)
  stage 2:  out[r, c] = max_i tpad[i + c, r] + w[i]     (slide along rows)

with w[k] = -(k - 50)^2 / (4 s) and sentinel (-1e30) padding instead of -inf.

Sharding: output rows are split across the 8 cores (13 rows each, 104 >= 101).
Each core runs both stages restricted to its 13 output rows -- no cross-core
communication. Stage 1 keeps input rows on partitions (101 used): one
broadcast-add (tensor_tensor over a [101, 13, 101] sliding-window AP) plus a
free-dim max-reduce. The [101, 13] result is transposed on the tensor engine,
sentinel-padded to [13, 224], and replicated into a [104, 128] layout
(partition P = cc*13 + r holds tpad[r, cc*13 : cc*13+128]) so stage 2 is
again one broadcast-add + free-dim max-reduce across 104 partitions.

Implementation is raw Bass (no Tile framework): manual semaphores avoid the
Tile entry/exit barrier overhead (~12 us on this toolchain), and all eight
replication gathers increment one shared semaphore so the single-sem-wait
ISA limit is satisfied with standalone wait instructions. The transpose
identity is built on-chip by gpsimd; w arrives pre-replicated from the host.
The replication gathers are spread over all three DMA issuers (SP HWDGE,
ACT HWDGE, and gpsimd SWDGE) so three descriptor generators run in parallel.
"""

import numpy as np

K = 101          # image/kernel size
PAD = 50
S = 13           # output rows per core
NCORES = 8
W = S + K - 1    # 113: window columns each core needs for compute
WT = 128         # transfer width: 512-byte rows
XCOLS = 224      # host-side padded row length (>= 7*13 + 128)
TCOLS = 224      # stage-2 padded t row length (>= 7*13 + 128)
SENT = np.float32(-1.0e30)

_CACHE = {}


def _build_nc():
    import concourse.bass as bass
    import concourse.mybir as mybir

    f32 = mybir.dt.float32
    add = mybir.AluOpType.add
    amax = mybir.AluOpType.max

    class _FastBass(bass.Bass):
        # Bass.__init__ ends with an all-engine barrier that separates the
        # const-tensor memsets from user code; this kernel uses none of the
        # const tensors and every cross-engine handoff is semaphore-guarded,
        # so the barrier only adds ~0.8 us of startup. Skip it during
        # construction only.
        def all_engine_barrier(self):
            if getattr(self, "_in_init", True):
                return None
            return super().all_engine_barrier()

    nc = _FastBass(target_bir_lowering=False, debug=False, enable_asserts=False)

    x_in = nc.dram_tensor("x", [K, WT], f32, kind="ExternalInput")
    w_in = nc.dram_tensor("w", [NCORES * S, K], f32, kind="ExternalInput")
    out = nc.dram_tensor("out", [NCORES * S, S], f32, kind="ExternalOutput")

    with (
        nc.sbuf_tensor("xs", [K, WT], f32) as xs,
        nc.sbuf_tensor("wsb", [NCORES * S, K], f32) as wsb,
        nc.sbuf_tensor("ones_k", [K, K], f32) as ones_k,
        nc.sbuf_tensor("idn", [K, K], f32) as idn,
        nc.sbuf_tensor("tmp1", [K, S * K], f32) as tmp1,
        nc.sbuf_tensor("t1", [K, S], f32) as t1,
        nc.sbuf_tensor("tpad", [S, TCOLS], f32) as tpad,
        nc.sbuf_tensor("X", [NCORES * S, WT], f32) as X,
        nc.sbuf_tensor("tmp2", [NCORES * S, S * K], f32) as tmp2,
        nc.sbuf_tensor("osb", [NCORES * S, S], f32) as osb,
        nc.psum_tensor("tp_ps", [S, K], f32) as tp_ps,
        nc.semaphore("s_dx") as s_dx,
        nc.semaphore("s_dw") as s_dw,
        nc.semaphore("s_idn") as s_idn,
        nc.semaphore("s_pe") as s_pe,
        nc.semaphore("s_dve") as s_dve,
        nc.semaphore("s_g") as s_g,
nc.semaphore("s_g2") as s_g2,
        nc.semaphore("s_out") as s_out,
        nc.Block() as block,
    ):
        xs_win = bass.AP(xs, 0, [[WT, K], [1, S], [1, K]])
        ws_b1 = bass.AP(wsb, 0, [[K, K], [0, S], [1, K]])
        tmp1_w = bass.AP(tmp1, 0, [[S * K, K], [K, S], [1, K]])
        X_win = bass.AP(X, 0, [[WT, NCORES * S], [1, S], [1, K]])
        ws_b2 = bass.AP(wsb, 0, [[K, NCORES * S], [0, S], [1, K]])
        tmp2_w = bass.AP(tmp2, 0, [[S * K, NCORES * S], [K, S], [1, K]])

        def gather(eng, cc, sem):
            return eng.dma_start(
                X[cc * S : (cc + 1) * S, :],
                tpad[0:S, cc * S : cc * S + WT],
                single_packet=True,
            ).then_inc(sem, 16)

        @block.sync
        def _(sync):
            sync.dma_start(
                bass.AP(xs, 0, [[WT, 51], [1, WT]]),
                bass.AP(x_in, 0, [[WT, 51], [1, WT]]),
            ).then_inc(s_dx, 16)
            sync.dma_start(
                bass.AP(xs, 51 * WT, [[WT, 50], [1, WT]]),
                bass.AP(x_in, 51 * WT, [[WT, 50], [1, WT]]),
            ).then_inc(s_dx, 16)
            sync.wait_ge(s_dve, 2)
            for cc in range(3):
                gather(sync, cc, s_g)

        @block.scalar
        def _(scalar):
            scalar.dma_start(wsb[:, :], w_in[:, :]).then_inc(s_dw, 16)
            scalar.wait_ge(s_dve, 2)
            for cc in range(3, 6):
                gather(scalar, cc, s_g)
            scalar.wait_ge(s_dve, 3)
            scalar.dma_start(out[:, :], osb[:, :]).then_inc(s_out, 16)

        @block.gpsimd
        def _(gpsimd):
            gpsimd.memset(ones_k[:, :], 1.0)
            gpsimd.drain()
            gpsimd.affine_select(
                idn[:, :],
                ones_k[:, :],
                [[1, K]],
                mybir.AluOpType.is_equal,
                0.0,
                base=0,
                channel_multiplier=-1,
            ).then_inc(s_idn, 1)
            gpsimd.wait_ge(s_dve, 2)
            for cc in range(6, NCORES):
                gather(gpsimd, cc, s_g2)

        @block.tensor
        def _(tensor):
            tensor.wait_ge(s_idn, 1)
            tensor.wait_ge(s_dve, 1)
            tensor.transpose(tp_ps[:, :], t1[:, :], idn[:, :]).then_inc(s_pe, 1)

        @block.vector
        def _(vector):
            vector.memset(tpad[:, :], float(SENT))
            vector.wait_ge(s_dw, 16)
            vector.wait_ge(s_dx, 32)
            # stage 1: tmp1[p, r, j] = xs[p, r + j] + w[j]
            vector.tensor_tensor(tmp1_w, xs_win, ws_b1, add)
            vector.drain()
            vector.tensor_reduce(
                t1[:, :], tmp1_w, axis=mybir.AxisListType.X, op=amax
            ).then_inc(s_dve, 1)
            vector.wait_ge(s_pe, 1)
            # tpad[r, 50 + p] = t1[p, r] (no drain needed: the WAW with the
            # early tpad memset is already separated by stage 1's drained pipe)
            vector.tensor_copy(tpad[0:S, PAD : PAD + K], tp_ps[:, :]).then_inc(
                s_dve, 1
            )
            vector.wait_ge(s_g, 96)
            vector.wait_ge(s_g2, 32)
            # stage 2: tmp2[P, c, i] = X[P, c + i] + w[i]
            vector.tensor_tensor(tmp2_w, X_win, ws_b2, add)
            vector.drain()
            vector.tensor_reduce(
                osb[:, :], tmp2_w, axis=mybir.AxisListType.X, op=amax
            ).then_inc(s_dve, 1)

    # restore normal barrier behavior for any framework-emitted code that
    # runs after the block (the skipped barriers are the init and block-exit
    # ones; the BSP postamble still drains all queues before NEFF end)
    nc._in_init = False
    return nc


def _prep_in_maps(input, scale):
    inp = np.asarray(input, dtype=np.float32)
    s = np.float32(np.asarray(scale).reshape(()))

    z = (np.arange(K, dtype=np.float32) - np.float32(PAD)).astype(np.float32)
    zsq = (z * z).astype(np.float32)
    wvec = (-zsq / (np.float32(4.0) * s)).astype(np.float32)
    w_rep = np.ascontiguousarray(np.tile(wvec[None, :], (NCORES * S, 1)))

    rowpad = np.full((K, XCOLS), SENT, dtype=np.float32)
    rowpad[:, PAD : PAD + K] = inp

    in_maps = []
    for k in range(NCORES):
        in_maps.append(
            {
                "x": np.ascontiguousarray(rowpad[:, S * k : S * k + WT]),
                "w": w_rep,
            }
        )
    return in_maps


def _unshard(results):
    out_full = np.empty((K, K), dtype=np.float32)
    for k, res in enumerate(results):
        o = np.asarray(res["out"]).reshape(NCORES, S, S)  # [cc, r_loc, c_in]
        block = o.transpose(1, 0, 2).reshape(S, NCORES * S)  # [r_loc, c]
        r0 = S * k
        nrows = min(S, K - r0)
        if nrows <= 0:
            continue
        out_full[r0 : r0 + nrows, :] = block[:nrows, :K]
    return out_full


def kernel(input, scale):
    from concourse.bass_utils import run_bass_kernel_spmd

    if "nc" not in _CACHE:
        _CACHE["nc"] = _build_nc()
    nc = _CACHE["nc"]

    in_maps = _prep_in_maps(input, scale)
    res = run_bass_kernel_spmd(nc, in_maps, core_ids=list(range(NCORES)))
    return _unshard(res.results)



# revision 11
# speedup vs baseline: 1.0696x; 1.0696x over previous
"""Trainium2 Bass kernel for nn_Dilation2D (101x101 grayscale dilation with a
parabolic structuring element).

Math: out[r, c] = max_{i,j} padded[i + c, j + r] + h[i, j] with
h[i, j] = -(z_i^2 + z_j^2) / (4 s) separable into f(i) + g(j), so the 2D
max-plus convolution factors into two 1D sliding passes:

  stage 1:  t[p, r] = max_j rowpad[p, j + r] + w[j]     (slide along columns)
  stage 2:  out[r, c] = max_i tpad[i + c, r] + w[i]     (slide along rows)

with w[k] = -(k - 50)^2 / (4 s) and sentinel (-60000, fp16-safe) padding.

Sharding: output rows are split across the 8 cores (13 rows each). Each core
runs both stages restricted to its 13 output rows -- no cross-core
communication.

V2 layout (vs the 24us f32 baseline):
  * all compute in fp16 (tolerance is 2e-2; winning max candidates carry
    ~5e-4 relative error in fp16) -- reduces DVE cycles (2x mode where the
    access pattern allows) and halves every DMA payload.
  * ONE input DMA: host packs x window (128 cols), replicated w row
    (102 cols, sentinel-terminated) into a single [104, 232] fp16 tensor.
  * ONE SBUF->SBUF gather DMA with a 3D access pattern replicates the
    transposed stage-1 result into the [104, 128] stage-2 layout
    (partition P = cc*13 + r takes tpad[r, cc*13 : cc*13+128]), replacing
    the baseline's 8 separate DMAs.
  * every DMA issues from the Sync engine's HWDGE: a single queue family
    keeps the compiler-generated end-of-NEFF queue-drain postamble short.
  * windows padded to 14 x 102 so reduce access patterns stay even-length
    (DVE 16-bit 2x mode needs stride-1/2-byte/aligned runs).
"""

import numpy as np

K = 101          # image/kernel size
PAD = 50
S = 13           # output rows per core
NCORES = 8
NP = NCORES * S  # 104
WT = 128         # x window columns held per partition
WIN = 14         # window positions computed per TT/RED (13 used + 1 pad)
VS1 = 10         # reduce windows handled by DVE (rest go to gpsimd)
JW = 102         # window length (101 used + 1 sentinel pad)
XWC = 232        # packed input row length: 128 x | 102 w | 2 pad
TPC = 232        # tpad row length (needs >= 7*13 + 128 = 219)
SENT = np.float16(-60000.0)

_CACHE = {}


def _build_nc():
    import concourse.bass as bass
    import concourse.mybir as mybir

    f16 = mybir.dt.float16
    add = mybir.AluOpType.add
    amax = mybir.AluOpType.max

    class _FastBass(bass.Bass):
        # Bass.__init__ ends with an all-engine barrier that separates the
        # const-tensor memsets from user code; this kernel uses none of the
        # const tensors and every cross-engine handoff is semaphore-guarded,
        # so the barrier only adds startup latency. Skip it during
        # construction only.
        def all_engine_barrier(self):
            if getattr(self, "_in_init", True):
                return None
            return super().all_engine_barrier()

    nc = _FastBass(target_bir_lowering=False, debug=False, enable_asserts=False)

    xw_d = nc.dram_tensor("xw", [NP, XWC], f16, kind="ExternalInput")
    out_d = nc.dram_tensor("out", [NP, WIN], f16, kind="ExternalOutput")

    from contextlib import ExitStack

    with ExitStack() as stack:
        ec = stack.enter_context
        xw = ec(nc.sbuf_tensor("xw_s", [NP, XWC], f16))
        ones_k = ec(nc.sbuf_tensor("ones_k", [K, K], f16))
        idn = ec(nc.sbuf_tensor("idn", [K, K], f16))
        tmp1 = ec(nc.sbuf_tensor("tmp1", [K, WIN * JW], f16))
        t1 = ec(nc.sbuf_tensor("t1", [K, WIN], f16))
        tpad = ec(nc.sbuf_tensor("tpad", [S, TPC], f16))
        X = ec(nc.sbuf_tensor("X", [NP, WT], f16))
        tmp2 = ec(nc.sbuf_tensor("tmp2", [NP, WIN * JW], f16))
        osb = ec(nc.sbuf_tensor("osb", [NP, WIN], f16))
        tp_ps = ec(nc.psum_tensor("tp_ps", [S, K], f16))
        s_in = ec(nc.semaphore("s_in"))
        s_idn = ec(nc.semaphore("s_idn"))
        s_tpm = ec(nc.semaphore("s_tpm"))
        s_tt1 = ec(nc.semaphore("s_tt1"))
        s_tt2 = ec(nc.semaphore("s_tt2"))
        s_t1 = ec(nc.semaphore("s_t1"))
        s_t1b = ec(nc.semaphore("s_t1b"))
        s_pe = ec(nc.semaphore("s_pe"))
        s_tp = ec(nc.semaphore("s_tp"))
        s_gx = ec(nc.semaphore("s_gx"))
        s_gx2 = ec(nc.semaphore("s_gx2"))
        s_o = ec(nc.semaphore("s_o"))
        s_ob = ec(nc.semaphore("s_ob"))
        s_out = ec(nc.semaphore("s_out"))
        block = ec(nc.Block())
        # stage 1: tmp1[p, rr, j] = xw[p, rr + j] + w[j]
        xw_win = bass.AP(xw, 0, [[XWC, K], [1, WIN], [1, JW]])
        w_b1 = bass.AP(xw, WT, [[XWC, K], [0, WIN], [1, JW]])
        tmp1_w = bass.AP(tmp1, 0, [[WIN * JW, K], [JW, WIN], [1, JW]])
        # stage 2: tmp2[P, c, i] = X[P, c + i] + w[i]
        X_win = bass.AP(X, 0, [[WT, NP], [1, WIN], [1, JW]])
        w_b2 = bass.AP(xw, WT, [[XWC, NP], [0, WIN], [1, JW]])
        tmp2_w = bass.AP(tmp2, 0, [[WIN * JW, NP], [JW, WIN], [1, JW]])

        def gather(eng, cc, sem):
            return eng.dma_start(
                X[cc * S : (cc + 1) * S, :],
                tpad[0:S, cc * S : cc * S + WT],
                single_packet=True,
            ).then_inc(sem, 16)

        @block.sync
        def _(sync):
            sync.dma_start(xw[:, :], xw_d[:, :]).then_inc(s_in, 16)
            sync.wait_ge(s_tp, 1)
            for cc in range(3):
                gather(sync, cc, s_gx)
            sync.wait_ge(s_o, 1)
            sync.dma_start(out_d[:, :], osb[:, :]).then_inc(s_out, 16)

        @block.scalar
        def _(scalar):
            scalar.wait_ge(s_tp, 1)
            for cc in range(3, 6):
                gather(scalar, cc, s_gx)

        @block.gpsimd
        def _(gpsimd):
            gpsimd.memset(tpad[:, :], float(SENT)).then_inc(s_tpm, 1)
            gpsimd.memset(ones_k[:, :], 1.0)
            gpsimd.drain()
            gpsimd.affine_select(
                idn[:, :],
                ones_k[:, :],
                [[1, K]],
                mybir.AluOpType.is_equal,
                0.0,
                base=0,
                channel_multiplier=-1,
            ).then_inc(s_idn, 1)
            gpsimd.wait_ge(s_tp, 1)
            for cc in range(6, NCORES):
                gather(gpsimd, cc, s_gx2)

        @block.tensor
        def _(tensor):
            tensor.wait_ge(s_idn, 1)
            tensor.wait_ge(s_t1, 1)
            tensor.transpose(tp_ps[:, :], t1[:, 0:S], idn[:, :]).then_inc(s_pe, 1)

        @block.vector
        def _(vector):
            vector.wait_ge(s_in, 16)
            vector.tensor_tensor(tmp1_w, xw_win, w_b1, add)
            vector.drain()
            vector.tensor_reduce(
                t1[:, :], tmp1_w, axis=mybir.AxisListType.X, op=amax
            ).then_inc(s_t1, 1)
            vector.wait_ge(s_pe, 1)
            vector.wait_ge(s_tpm, 1)
            # tpad[r, 50 + p] = t1[p, r]
            vector.tensor_copy(tpad[0:S, PAD : PAD + K], tp_ps[:, :]).then_inc(
                s_tp, 1
            )
            vector.wait_ge(s_gx, 96)
            vector.wait_ge(s_gx2, 32)
            vector.tensor_tensor(tmp2_w, X_win, w_b2, add)
            vector.drain()
            vector.tensor_reduce(
                osb[:, :], tmp2_w, axis=mybir.AxisListType.X, op=amax
            ).then_inc(s_o, 1)

    nc._in_init = False
    return nc


def _prep_in_maps(input, scale):
    inp = np.asarray(input, dtype=np.float32)
    s = np.float32(np.asarray(scale).reshape(()))

    z = (np.arange(K, dtype=np.float32) - np.float32(PAD)).astype(np.float32)
    wvec = (-(z * z) / (np.float32(4.0) * s)).astype(np.float16)
    w102 = np.full(JW, SENT, dtype=np.float16)
    w102[:K] = wvec

    rowpad = np.full((K, 224), SENT, dtype=np.float16)
    rowpad[:, PAD : PAD + K] = inp.astype(np.float16)

    in_maps = []
    for k in range(NCORES):
        xw = np.full((NP, XWC), SENT, dtype=np.float16)
        xw[:K, :WT] = rowpad[:, S * k : S * k + WT]
        xw[:, WT : WT + JW] = w102[None, :]
        in_maps.append({"xw": np.ascontiguousarray(xw)})
    return in_maps


def _unshard(results):
    out_full = np.empty((K, K), dtype=np.float32)
    for k, res in enumerate(results):
        o = np.asarray(res["out"]).astype(np.float32)  # [104, 14]
        blk = o.reshape(NCORES, S, WIN)[:, :, :S]  # [cc, r_loc, c_loc]
        blk = blk.transpose(1, 0, 2).reshape(S, NP)  # [r_loc, c]
        r0 = S * k
        nrows = min(S, K - r0)
        if nrows <= 0:
            continue
        out_full[r0 : r0 + nrows, :] = blk[:nrows, :K]
    return out_full


def kernel(input, scale):
    from concourse.bass_utils import run_bass_kernel_spmd

    if "nc" not in _CACHE:
        _CACHE["nc"] = _build_nc()
    nc = _CACHE["nc"]

    in_maps = _prep_in_maps(input, scale)
    res = run_bass_kernel_spmd(nc, in_maps, core_ids=list(range(NCORES)))
    return _unshard(res.results)


# revision 13
# speedup vs baseline: 1.0712x; 1.0015x over previous
"""Trainium2 Bass kernel for nn_Dilation2D (101x101 grayscale dilation with a
parabolic structuring element).

Math: out[r, c] = max_{i,j} padded[i + c, j + r] + h[i, j] with
h[i, j] = -(z_i^2 + z_j^2) / (4 s) separable into f(i) + g(j), so the 2D
max-plus convolution factors into two 1D sliding passes:

  stage 1:  t[p, r] = max_j rowpad[p, j + r] + w[j]     (slide along columns)
  stage 2:  out[r, c] = max_i tpad[i + c, r] + w[i]     (slide along rows)

with w[k] = -(k - 50)^2 / (4 s) and sentinel (-60000, fp16-safe) padding.

Sharding: output rows are split across the 8 cores (13 rows each). Each core
runs both stages restricted to its 13 output rows -- no cross-core
communication.

V2 layout (vs the 24us f32 baseline):
  * all compute in fp16 (tolerance is 2e-2; winning max candidates carry
    ~5e-4 relative error in fp16) -- reduces DVE cycles (2x mode where the
    access pattern allows) and halves every DMA payload.
  * ONE input DMA: host packs x window (128 cols), replicated w row
    (102 cols, sentinel-terminated) into a single [104, 232] fp16 tensor.
  * ONE SBUF->SBUF gather DMA with a 3D access pattern replicates the
    transposed stage-1 result into the [104, 128] stage-2 layout
    (partition P = cc*13 + r takes tpad[r, cc*13 : cc*13+128]), replacing
    the baseline's 8 separate DMAs.
  * every DMA issues from the Sync engine's HWDGE: a single queue family
    keeps the compiler-generated end-of-NEFF queue-drain postamble short.
  * windows padded to 14 x 102 so reduce access patterns stay even-length
    (DVE 16-bit 2x mode needs stride-1/2-byte/aligned runs).
"""

import numpy as np

K = 101          # image/kernel size
PAD = 50
S = 13           # output rows per core
NCORES = 8
NP = NCORES * S  # 104
WT = 128         # x window columns held per partition
WIN = 14         # window positions computed per TT/RED (13 used + 1 pad)
VS1 = 10         # reduce windows handled by DVE (rest go to gpsimd)
JW = 102         # window length (101 used + 1 sentinel pad)
JH = 52          # folded half-window length (even, keeps 2x mode)
XWC = 232        # packed input row length: 128 x | 102 w | 2 pad
TPC = 232        # tpad row length (needs >= 7*13 + 128 = 219)
SENT = np.float16(-60000.0)

_CACHE = {}


def _build_nc():
    import concourse.bass as bass
    import concourse.mybir as mybir

    f16 = mybir.dt.float16
    add = mybir.AluOpType.add
    amax = mybir.AluOpType.max

    class _FastBass(bass.Bass):
        # Bass.__init__ ends with an all-engine barrier that separates the
        # const-tensor memsets from user code; this kernel uses none of the
        # const tensors and every cross-engine handoff is semaphore-guarded,
        # so the barrier only adds startup latency. Skip it during
        # construction only.
        def all_engine_barrier(self):
            if getattr(self, "_in_init", True):
                return None
            return super().all_engine_barrier()

    nc = _FastBass(target_bir_lowering=False, debug=False, enable_asserts=False)

    xw_d = nc.dram_tensor("xw", [NP, XWC], f16, kind="ExternalInput")
    out_d = nc.dram_tensor("out", [NP, WIN], f16, kind="ExternalOutput")

    from contextlib import ExitStack

    with ExitStack() as stack:
        ec = stack.enter_context
        xw = ec(nc.sbuf_tensor("xw_s", [NP, XWC], f16))
        ones_k = ec(nc.sbuf_tensor("ones_k", [K, K], f16))
        idn = ec(nc.sbuf_tensor("idn", [K, K], f16))
        tmp1 = ec(nc.sbuf_tensor("tmp1", [K, WIN * JW], f16))
        t1 = ec(nc.sbuf_tensor("t1", [K, WIN], f16))
        tpad = ec(nc.sbuf_tensor("tpad", [S, TPC], f16))
        X = ec(nc.sbuf_tensor("X", [NP, WT], f16))
        tmp2 = ec(nc.sbuf_tensor("tmp2", [NP, WIN * JW], f16))
        tmp1h = ec(nc.sbuf_tensor("tmp1h", [K, WIN * JH], f16))
        tmp2h = ec(nc.sbuf_tensor("tmp2h", [NP, WIN * JH], f16))
        warm = ec(nc.sbuf_tensor("warm", [1, 16], f16))
        osb = ec(nc.sbuf_tensor("osb", [NP, WIN], f16))
        tp_ps = ec(nc.psum_tensor("tp_ps", [S, K], f16))
        s_in = ec(nc.semaphore("s_in"))
        s_idn = ec(nc.semaphore("s_idn"))
        s_tpm = ec(nc.semaphore("s_tpm"))
        s_tt1 = ec(nc.semaphore("s_tt1"))
        s_tt2 = ec(nc.semaphore("s_tt2"))
        s_t1 = ec(nc.semaphore("s_t1"))
        s_t1b = ec(nc.semaphore("s_t1b"))
        s_pe = ec(nc.semaphore("s_pe"))
        s_tp = ec(nc.semaphore("s_tp"))
        s_gx = ec(nc.semaphore("s_gx"))
        s_gx2 = ec(nc.semaphore("s_gx2"))
        s_o = ec(nc.semaphore("s_o"))
        s_ob = ec(nc.semaphore("s_ob"))
        s_out = ec(nc.semaphore("s_out"))
        s_warm = ec(nc.semaphore("s_warm"))
        block = ec(nc.Block())
        # stage 1: tmp1[p, rr, j] = xw[p, rr + j] + w[j]
        xw_win = bass.AP(xw, 0, [[XWC, K], [1, WIN], [1, JW]])
        w_b1 = bass.AP(xw, WT, [[XWC, K], [0, WIN], [1, JW]])
        tmp1_w = bass.AP(tmp1, 0, [[WIN * JW, K], [JW, WIN], [1, JW]])
        # stage 2: tmp2[P, c, i] = X[P, c + i] + w[i]
        X_win = bass.AP(X, 0, [[WT, NP], [1, WIN], [1, JW]])
        w_b2 = bass.AP(xw, WT, [[XWC, NP], [0, WIN], [1, JW]])
        tmp2_w = bass.AP(tmp2, 0, [[WIN * JW, NP], [JW, WIN], [1, JW]])
        # fold-in-half max: h[p, rr, j'] = max(tmp[p, rr, j'], tmp[p, rr, j'+50])
        # (j' in 0..51 covers 0..51 and 50..101; overlap is harmless for max,
        # and the 52-long even runs keep the DVE 16-bit 2x mode on)
        tmp1_l = bass.AP(tmp1, 0, [[WIN * JW, K], [JW, WIN], [1, JH]])
        tmp1_r = bass.AP(tmp1, JW - JH, [[WIN * JW, K], [JW, WIN], [1, JH]])
        tmp1h_w = bass.AP(tmp1h, 0, [[WIN * JH, K], [JH, WIN], [1, JH]])
        tmp2_l = bass.AP(tmp2, 0, [[WIN * JW, NP], [JW, WIN], [1, JH]])
        tmp2_r = bass.AP(tmp2, JW - JH, [[WIN * JW, NP], [JW, WIN], [1, JH]])
        tmp2h_w = bass.AP(tmp2h, 0, [[WIN * JH, NP], [JH, WIN], [1, JH]])

        def gather(eng, cc, sem):
            return eng.dma_start(
                X[cc * S : (cc + 1) * S, :],
                tpad[0:S, cc * S : cc * S + WT],
                single_packet=True,
            ).then_inc(sem, 16)

        @block.sync
        def _(sync):
            sync.dma_start(xw[:, :], xw_d[:, :]).then_inc(s_in, 16)
            sync.wait_ge(s_tp, 1)
            for cc in range(3):
                gather(sync, cc, s_gx)
            sync.wait_ge(s_o, 1)
            sync.dma_start(out_d[:, :], osb[:, :]).then_inc(s_out, 16)

        @block.scalar
        def _(scalar):
            # first DMA on a cold HWDGE costs ~800ns extra (measured); warm
            # the ACT path with a throwaway 1-descriptor load at t=0 so the
            # mid-kernel gathers issue at full speed.
            scalar.dma_start(warm[:, :], xw_d[0:1, 0:16]).then_inc(s_warm, 16)
            scalar.wait_ge(s_tp, 1)
            for cc in range(3, 6):
                gather(scalar, cc, s_gx)

        @block.gpsimd
        def _(gpsimd):
            gpsimd.memset(tpad[:, :], float(SENT)).then_inc(s_tpm, 1)
            gpsimd.memset(ones_k[:, :], 1.0)
            gpsimd.drain()
            gpsimd.affine_select(
                idn[:, :],
                ones_k[:, :],
                [[1, K]],
                mybir.AluOpType.is_equal,
                0.0,
                base=0,
                channel_multiplier=-1,
            ).then_inc(s_idn, 1)
            gpsimd.wait_ge(s_tp, 1)
            for cc in range(6, NCORES):
                gather(gpsimd, cc, s_gx2)

        @block.tensor
        def _(tensor):
            tensor.wait_ge(s_idn, 1)
            tensor.wait_ge(s_t1, 1)
            tensor.transpose(tp_ps[:, :], t1[:, 0:S], idn[:, :]).then_inc(s_pe, 1)

        @block.vector
        def _(vector):
            vector.wait_ge(s_in, 16)
            vector.tensor_tensor(tmp1_w, xw_win, w_b1, add)
            vector.drain()
            vector.tensor_tensor(tmp1h_w, tmp1_l, tmp1_r, amax)
            vector.drain()
            vector.tensor_reduce(
                t1[:, :], tmp1h_w, axis=mybir.AxisListType.X, op=amax
            ).then_inc(s_t1, 1)
            vector.wait_ge(s_pe, 1)
            vector.wait_ge(s_tpm, 1)
            # tpad[r, 50 + p] = t1[p, r]
            vector.tensor_copy(tpad[0:S, PAD : PAD + K], tp_ps[:, :]).then_inc(
                s_tp, 1
            )
            vector.wait_ge(s_gx, 96)
            vector.wait_ge(s_gx2, 32)
            vector.tensor_tensor(tmp2_w, X_win, w_b2, add)
            vector.drain()
            vector.tensor_tensor(tmp2h_w, tmp2_l, tmp2_r, amax)
            vector.drain()
            vector.tensor_reduce(
                osb[:, :], tmp2h_w, axis=mybir.AxisListType.X, op=amax
            ).then_inc(s_o, 1)

    nc._in_init = False
    return nc


def _prep_in_maps(input, scale):
    inp = np.asarray(input, dtype=np.float32)
    s = np.float32(np.asarray(scale).reshape(()))

    z = (np.arange(K, dtype=np.float32) - np.float32(PAD)).astype(np.float32)
    wvec = (-(z * z) / (np.float32(4.0) * s)).astype(np.float16)
    w102 = np.full(JW, SENT, dtype=np.float16)
    w102[:K] = wvec

    rowpad = np.full((K, 224), SENT, dtype=np.float16)
    rowpad[:, PAD : PAD + K] = inp.astype(np.float16)

    in_maps = []
    for k in range(NCORES):
        xw = np.full((NP, XWC), SENT, dtype=np.float16)
        xw[:K, :WT] = rowpad[:, S * k : S * k + WT]
        xw[:, WT : WT + JW] = w102[None, :]
        in_maps.append({"xw": np.ascontiguousarray(xw)})
    return in_maps


def _unshard(results):
    out_full = np.empty((K, K), dtype=np.float32)
    for k, res in enumerate(results):
        o = np.asarray(res["out"]).astype(np.float32)  # [104, 14]
        blk = o.reshape(NCORES, S, WIN)[:, :, :S]  # [cc, r_loc, c_loc]
        blk = blk.transpose(1, 0, 2).reshape(S, NP)  # [r_loc, c]
        r0 = S * k
        nrows = min(S, K - r0)
        if nrows <= 0:
            continue
        out_full[r0 : r0 + nrows, :] = blk[:nrows, :K]
    return out_full


def kernel(input, scale):
    from concourse.bass_utils import run_bass_kernel_spmd

    if "nc" not in _CACHE:
        _CACHE["nc"] = _build_nc()
    nc = _CACHE["nc"]

    in_maps = _prep_in_maps(input, scale)
    res = run_bass_kernel_spmd(nc, in_maps, core_ids=list(range(NCORES)))
    return _unshard(res.results)


# revision 14
# speedup vs baseline: 1.1005x; 1.0273x over previous
"""Trainium2 Bass kernel for nn_Dilation2D (101x101 grayscale dilation with a
parabolic structuring element).

Math: out[r, c] = max_{i,j} padded[i + c, j + r] + h[i, j] with
h[i, j] = -(z_i^2 + z_j^2) / (4 s) separable into f(i) + g(j), so the 2D
max-plus convolution factors into two 1D sliding passes:

  stage 1:  t[p, r] = max_j rowpad[p, j + r] + w[j]     (slide along columns)
  stage 2:  out[r, c] = max_i tpad[i + c, r] + w[i]     (slide along rows)

with w[k] = -(k - 50)^2 / (4 s) and sentinel (-60000, fp16-safe) padding.

Sharding: output rows are split across the 8 cores (13 rows each). Each core
runs both stages restricted to its 13 output rows -- no cross-core
communication.

V2 layout (vs the 24us f32 baseline):
  * all compute in fp16 (tolerance is 2e-2; winning max candidates carry
    ~5e-4 relative error in fp16) -- reduces DVE cycles (2x mode where the
    access pattern allows) and halves every DMA payload.
  * ONE input DMA: host packs x window (128 cols), replicated w row
    (102 cols, sentinel-terminated) into a single [104, 232] fp16 tensor.
  * ONE SBUF->SBUF gather DMA with a 3D access pattern replicates the
    transposed stage-1 result into the [104, 128] stage-2 layout
    (partition P = cc*13 + r takes tpad[r, cc*13 : cc*13+128]), replacing
    the baseline's 8 separate DMAs.
  * every DMA issues from the Sync engine's HWDGE: a single queue family
    keeps the compiler-generated end-of-NEFF queue-drain postamble short.
  * windows padded to 14 x 102 so reduce access patterns stay even-length
    (DVE 16-bit 2x mode needs stride-1/2-byte/aligned runs).
"""

import numpy as np

K = 101          # image/kernel size
PAD = 50
S = 13           # output rows per core
NCORES = 8
NP = NCORES * S  # 104
WT = 128         # x window columns held per partition
WIN = 14         # window positions computed per TT/RED (13 used + 1 pad)
VS1 = 10         # reduce windows handled by DVE (rest go to gpsimd)
JW = 102         # window length (101 used + 1 sentinel pad)
JH = 52          # folded half-window length (even, keeps 2x mode)
XWC = 232        # packed input row length: 128 x | 102 w | 2 pad
TPC = 232        # tpad row length (needs >= 7*13 + 128 = 219)
SENT = np.float16(-60000.0)

_CACHE = {}


def _build_nc():
    import concourse.bass as bass
    import concourse.mybir as mybir

    f16 = mybir.dt.float16
    add = mybir.AluOpType.add
    amax = mybir.AluOpType.max

    class _FastBass(bass.Bass):
        # Bass.__init__ ends with an all-engine barrier that separates the
        # const-tensor memsets from user code; this kernel uses none of the
        # const tensors and every cross-engine handoff is semaphore-guarded,
        # so the barrier only adds startup latency. Skip it during
        # construction only.
        def all_engine_barrier(self):
            if getattr(self, "_in_init", True):
                return None
            return super().all_engine_barrier()

    nc = _FastBass(target_bir_lowering=False, debug=False, enable_asserts=False)

    xw_d = nc.dram_tensor("xw", [NP, XWC], f16, kind="ExternalInput")
    out_d = nc.dram_tensor("out", [NP, WIN], f16, kind="ExternalOutput")

    from contextlib import ExitStack

    with ExitStack() as stack:
        ec = stack.enter_context
        xw = ec(nc.sbuf_tensor("xw_s", [NP, XWC], f16))
        ones_k = ec(nc.sbuf_tensor("ones_k", [K, K], f16))
        idn = ec(nc.sbuf_tensor("idn", [K, K], f16))
        tmp1 = ec(nc.sbuf_tensor("tmp1", [K, WIN * JW], f16))
        t1 = ec(nc.sbuf_tensor("t1", [K, WIN], f16))
        tpad = ec(nc.sbuf_tensor("tpad", [S, TPC], f16))
        X = ec(nc.sbuf_tensor("X", [NP, WT], f16))
        tmp2 = ec(nc.sbuf_tensor("tmp2", [NP, WIN * JW], f16))
        tmp1h = ec(nc.sbuf_tensor("tmp1h", [K, WIN * JH], f16))
        tmp2h = ec(nc.sbuf_tensor("tmp2h", [NP, WIN * JH], f16))
        osb = ec(nc.sbuf_tensor("osb", [NP, WIN], f16))
        tp_ps = ec(nc.psum_tensor("tp_ps", [S, K], f16))
        s_in = ec(nc.semaphore("s_in"))
        s_idn = ec(nc.semaphore("s_idn"))
        s_tpm = ec(nc.semaphore("s_tpm"))
        s_tt1 = ec(nc.semaphore("s_tt1"))
        s_tt2 = ec(nc.semaphore("s_tt2"))
        s_t1 = ec(nc.semaphore("s_t1"))
        s_t1b = ec(nc.semaphore("s_t1b"))
        s_pe = ec(nc.semaphore("s_pe"))
        s_tp = ec(nc.semaphore("s_tp"))
        s_gx = ec(nc.semaphore("s_gx"))
        s_gx2 = ec(nc.semaphore("s_gx2"))
        s_o = ec(nc.semaphore("s_o"))
        s_ob = ec(nc.semaphore("s_ob"))
        s_out = ec(nc.semaphore("s_out"))
        block = ec(nc.Block())
        # stage 1: tmp1[p, rr, j] = xw[p, rr + j] + w[j]
        xw_win = bass.AP(xw, 0, [[XWC, K], [1, WIN], [1, JW]])
        w_b1 = bass.AP(xw, WT, [[XWC, K], [0, WIN], [1, JW]])
        tmp1_w = bass.AP(tmp1, 0, [[WIN * JW, K], [JW, WIN], [1, JW]])
        # stage 2: tmp2[P, c, i] = X[P, c + i] + w[i]
        X_win = bass.AP(X, 0, [[WT, NP], [1, WIN], [1, JW]])
        w_b2 = bass.AP(xw, WT, [[XWC, NP], [0, WIN], [1, JW]])
        tmp2_w = bass.AP(tmp2, 0, [[WIN * JW, NP], [JW, WIN], [1, JW]])
        # fold-in-half max: h[p, rr, j'] = max(tmp[p, rr, j'], tmp[p, rr, j'+50])
        # (j' in 0..51 covers 0..51 and 50..101; overlap is harmless for max,
        # and the 52-long even runs keep the DVE 16-bit 2x mode on)
        tmp1_l = bass.AP(tmp1, 0, [[WIN * JW, K], [JW, WIN], [1, JH]])
        tmp1_r = bass.AP(tmp1, JW - JH, [[WIN * JW, K], [JW, WIN], [1, JH]])
        tmp1h_w = bass.AP(tmp1h, 0, [[WIN * JH, K], [JH, WIN], [1, JH]])
        tmp2_l = bass.AP(tmp2, 0, [[WIN * JW, NP], [JW, WIN], [1, JH]])
        tmp2_r = bass.AP(tmp2, JW - JH, [[WIN * JW, NP], [JW, WIN], [1, JH]])
        tmp2h_w = bass.AP(tmp2h, 0, [[WIN * JH, NP], [JH, WIN], [1, JH]])

        def gather(eng, cc, sem):
            return eng.dma_start(
                X[cc * S : (cc + 1) * S, :],
                tpad[0:S, cc * S : cc * S + WT],
                single_packet=True,
            ).then_inc(sem, 16)

        @block.sync
        def _(sync):
            sync.dma_start(xw[:, :], xw_d[:, :]).then_inc(s_in, 16)
            sync.wait_ge(s_tp, 1)
            for cc in range(3):
                gather(sync, cc, s_gx)
            sync.wait_ge(s_o, 1)
            sync.dma_start(out_d[:, :], osb[:, :]).then_inc(s_out, 16)

        @block.scalar
        def _(scalar):
            # the HWDGE is one shared device (~560ns per DMA regardless of
            # issuing engine), so SP+ACT together get 5 gathers and the
            # independent SWDGE (gpsimd) takes the other 3.
            scalar.wait_ge(s_tp, 1)
            for cc in range(3, 5):
                gather(scalar, cc, s_gx)

        @block.gpsimd
        def _(gpsimd):
            gpsimd.memset(tpad[:, :], float(SENT)).then_inc(s_tpm, 1)
            gpsimd.memset(ones_k[:, :], 1.0)
            gpsimd.drain()
            gpsimd.affine_select(
                idn[:, :],
                ones_k[:, :],
                [[1, K]],
                mybir.AluOpType.is_equal,
                0.0,
                base=0,
                channel_multiplier=-1,
            ).then_inc(s_idn, 1)
            # pre-wake on the transpose sem so the Q7 is already spinning on
            # s_tp when it fires (cuts ~0.8us of gpsimd wake latency)
            gpsimd.wait_ge(s_pe, 1)
            gpsimd.wait_ge(s_tp, 1)
            for cc in range(5, NCORES):
                gather(gpsimd, cc, s_gx2)

        @block.tensor
        def _(tensor):
            tensor.wait_ge(s_idn, 1)
            tensor.wait_ge(s_t1, 1)
            tensor.transpose(tp_ps[:, :], t1[:, 0:S], idn[:, :]).then_inc(s_pe, 1)

        @block.vector
        def _(vector):
            vector.wait_ge(s_in, 16)
            vector.tensor_tensor(tmp1_w, xw_win, w_b1, add)
            vector.drain()
            vector.tensor_tensor(tmp1h_w, tmp1_l, tmp1_r, amax)
            vector.drain()
            vector.tensor_reduce(
                t1[:, :], tmp1h_w, axis=mybir.AxisListType.X, op=amax
            ).then_inc(s_t1, 1)
            vector.wait_ge(s_pe, 1)
            vector.wait_ge(s_tpm, 1)
            # tpad[r, 50 + p] = t1[p, r]
            vector.tensor_copy(tpad[0:S, PAD : PAD + K], tp_ps[:, :]).then_inc(
                s_tp, 1
            )
            vector.wait_ge(s_gx, 80)
            vector.wait_ge(s_gx2, 48)
            vector.tensor_tensor(tmp2_w, X_win, w_b2, add)
            vector.drain()
            vector.tensor_tensor(tmp2h_w, tmp2_l, tmp2_r, amax)
            vector.drain()
            vector.tensor_reduce(
                osb[:, :], tmp2h_w, axis=mybir.AxisListType.X, op=amax
            ).then_inc(s_o, 1)

    nc._in_init = False
    return nc


def _prep_in_maps(input, scale):
    inp = np.asarray(input, dtype=np.float32)
    s = np.float32(np.asarray(scale).reshape(()))

    z = (np.arange(K, dtype=np.float32) - np.float32(PAD)).astype(np.float32)
    wvec = (-(z * z) / (np.float32(4.0) * s)).astype(np.float16)
    w102 = np.full(JW, SENT, dtype=np.float16)
    w102[:K] = wvec

    rowpad = np.full((K, 224), SENT, dtype=np.float16)
    rowpad[:, PAD : PAD + K] = inp.astype(np.float16)

    in_maps = []
    for k in range(NCORES):
        xw = np.full((NP, XWC), SENT, dtype=np.float16)
        xw[:K, :WT] = rowpad[:, S * k : S * k + WT]
        xw[:, WT : WT + JW] = w102[None, :]
        in_maps.append({"xw": np.ascontiguousarray(xw)})
    return in_maps


def _unshard(results):
    out_full = np.empty((K, K), dtype=np.float32)
    for k, res in enumerate(results):
        o = np.asarray(res["out"]).astype(np.float32)  # [104, 14]
        blk = o.reshape(NCORES, S, WIN)[:, :, :S]  # [cc, r_loc, c_loc]
        blk = blk.transpose(1, 0, 2).reshape(S, NP)  # [r_loc, c]
        r0 = S * k
        nrows = min(S, K - r0)
        if nrows <= 0:
            continue
        out_full[r0 : r0 + nrows, :] = blk[:nrows, :K]
    return out_full


def kernel(input, scale):
    from concourse.bass_utils import run_bass_kernel_spmd

    if "nc" not in _CACHE:
        _CACHE["nc"] = _build_nc()
    nc = _CACHE["nc"]

    in_maps = _prep_in_maps(input, scale)
    res = run_bass_kernel_spmd(nc, in_maps, core_ids=list(range(NCORES)))
    return _unshard(res.results)


# revision 15
# speedup vs baseline: 1.1070x; 1.0060x over previous
"""Trainium2 Bass kernel for nn_Dilation2D (101x101 grayscale dilation with a
parabolic structuring element).

Math: out[r, c] = max_{i,j} padded[i + c, j + r] + h[i, j] with
h[i, j] = -(z_i^2 + z_j^2) / (4 s) separable into f(i) + g(j), so the 2D
max-plus convolution factors into two 1D sliding passes:

  stage 1:  t[p, r] = max_j rowpad[p, j + r] + w[j]     (slide along columns)
  stage 2:  out[r, c] = max_i tpad[i + c, r] + w[i]     (slide along rows)

with w[k] = -(k - 50)^2 / (4 s) and sentinel (-60000, fp16-safe) padding.

Sharding: output rows are split across the 8 cores (13 rows each). Each core
runs both stages restricted to its 13 output rows -- no cross-core
communication.

V2 layout (vs the 24us f32 baseline):
  * all compute in fp16 (tolerance is 2e-2; winning max candidates carry
    ~5e-4 relative error in fp16) -- reduces DVE cycles (2x mode where the
    access pattern allows) and halves every DMA payload.
  * ONE input DMA: host packs x window (128 cols), replicated w row
    (102 cols, sentinel-terminated) into a single [104, 232] fp16 tensor.
  * ONE SBUF->SBUF gather DMA with a 3D access pattern replicates the
    transposed stage-1 result into the [104, 128] stage-2 layout
    (partition P = cc*13 + r takes tpad[r, cc*13 : cc*13+128]), replacing
    the baseline's 8 separate DMAs.
  * every DMA issues from the Sync engine's HWDGE: a single queue family
    keeps the compiler-generated end-of-NEFF queue-drain postamble short.
  * windows padded to 14 x 102 so reduce access patterns stay even-length
    (DVE 16-bit 2x mode needs stride-1/2-byte/aligned runs).
"""

import numpy as np

K = 101          # image/kernel size
PAD = 50
S = 13           # output rows per core
NCORES = 8
NP = NCORES * S  # 104
WT = 128         # x window columns held per partition
WIN = 14         # window positions computed per TT/RED (13 used + 1 pad)
VS1 = 10         # reduce windows handled by DVE (rest go to gpsimd)
JW = 102         # window length (101 used + 1 sentinel pad)
JH = 52          # folded half-window length (even, keeps 2x mode)
XWC = 256        # packed input row length: 128 x | 102 w | pad (512B rows)
TPC = 232        # tpad row length (needs >= 7*13 + 128 = 219)
SENT = np.float16(-60000.0)

_CACHE = {}


def _build_nc():
    import concourse.bass as bass
    import concourse.mybir as mybir

    f16 = mybir.dt.float16
    add = mybir.AluOpType.add
    amax = mybir.AluOpType.max

    class _FastBass(bass.Bass):
        # Bass.__init__ ends with an all-engine barrier that separates the
        # const-tensor memsets from user code; this kernel uses none of the
        # const tensors and every cross-engine handoff is semaphore-guarded,
        # so the barrier only adds startup latency. Skip it during
        # construction only.
        def all_engine_barrier(self):
            if getattr(self, "_in_init", True):
                return None
            return super().all_engine_barrier()

    nc = _FastBass(target_bir_lowering=False, debug=False, enable_asserts=False)

    xw_d = nc.dram_tensor("xw", [NP, XWC], f16, kind="ExternalInput")
    out_d = nc.dram_tensor("out", [NP, WIN], f16, kind="ExternalOutput")

    from contextlib import ExitStack

    with ExitStack() as stack:
        ec = stack.enter_context
        xw = ec(nc.sbuf_tensor("xw_s", [NP, XWC], f16))
        ones_k = ec(nc.sbuf_tensor("ones_k", [K, K], f16))
        idn = ec(nc.sbuf_tensor("idn", [K, K], f16))
        tmp1 = ec(nc.sbuf_tensor("tmp1", [K, WIN * JW], f16))
        t1 = ec(nc.sbuf_tensor("t1", [K, WIN], f16))
        tpad = ec(nc.sbuf_tensor("tpad", [S, TPC], f16))
        X = ec(nc.sbuf_tensor("X", [NP, WT], f16))
        tmp2 = ec(nc.sbuf_tensor("tmp2", [NP, WIN * JW], f16))
        tmp1h = ec(nc.sbuf_tensor("tmp1h", [K, WIN * JH], f16))
        tmp2h = ec(nc.sbuf_tensor("tmp2h", [NP, WIN * JH], f16))
        osb = ec(nc.sbuf_tensor("osb", [NP, WIN], f16))
        tp_ps = ec(nc.psum_tensor("tp_ps", [S, K], f16))
        s_in = ec(nc.semaphore("s_in"))
        s_idn = ec(nc.semaphore("s_idn"))
        s_tpm = ec(nc.semaphore("s_tpm"))
        s_tt1 = ec(nc.semaphore("s_tt1"))
        s_tt2 = ec(nc.semaphore("s_tt2"))
        s_t1 = ec(nc.semaphore("s_t1"))
        s_t1b = ec(nc.semaphore("s_t1b"))
        s_pe = ec(nc.semaphore("s_pe"))
        s_tp = ec(nc.semaphore("s_tp"))
        s_gx = ec(nc.semaphore("s_gx"))
        s_gx2 = ec(nc.semaphore("s_gx2"))
        s_o = ec(nc.semaphore("s_o"))
        s_ob = ec(nc.semaphore("s_ob"))
        s_out = ec(nc.semaphore("s_out"))
        block = ec(nc.Block())
        # stage 1: tmp1[p, rr, j] = xw[p, rr + j] + w[j]
        xw_win = bass.AP(xw, 0, [[XWC, K], [1, WIN], [1, JW]])
        w_b1 = bass.AP(xw, WT, [[XWC, K], [0, WIN], [1, JW]])
        tmp1_w = bass.AP(tmp1, 0, [[WIN * JW, K], [JW, WIN], [1, JW]])
        # stage 2: tmp2[P, c, i] = X[P, c + i] + w[i]
        X_win = bass.AP(X, 0, [[WT, NP], [1, WIN], [1, JW]])
        w_b2 = bass.AP(xw, WT, [[XWC, NP], [0, WIN], [1, JW]])
        tmp2_w = bass.AP(tmp2, 0, [[WIN * JW, NP], [JW, WIN], [1, JW]])
        # fold-in-half max: h[p, rr, j'] = max(tmp[p, rr, j'], tmp[p, rr, j'+50])
        # (j' in 0..51 covers 0..51 and 50..101; overlap is harmless for max,
        # and the 52-long even runs keep the DVE 16-bit 2x mode on)
        tmp1_l = bass.AP(tmp1, 0, [[WIN * JW, K], [JW, WIN], [1, JH]])
        tmp1_r = bass.AP(tmp1, JW - JH, [[WIN * JW, K], [JW, WIN], [1, JH]])
        tmp1h_w = bass.AP(tmp1h, 0, [[WIN * JH, K], [JH, WIN], [1, JH]])
        tmp2_l = bass.AP(tmp2, 0, [[WIN * JW, NP], [JW, WIN], [1, JH]])
        tmp2_r = bass.AP(tmp2, JW - JH, [[WIN * JW, NP], [JW, WIN], [1, JH]])
        tmp2h_w = bass.AP(tmp2h, 0, [[WIN * JH, NP], [JH, WIN], [1, JH]])

        def gather(eng, cc, sem):
            return eng.dma_start(
                X[cc * S : (cc + 1) * S, :],
                tpad[0:S, cc * S : cc * S + WT],
                single_packet=True,
            ).then_inc(sem, 16)

        @block.sync
        def _(sync):
            sync.dma_start(xw[:, :], xw_d[:, :]).then_inc(s_in, 16)
            sync.wait_ge(s_tp, 1)
            for cc in range(3):
                gather(sync, cc, s_gx)
            sync.wait_ge(s_o, 1)
            sync.dma_start(out_d[:, :], osb[:, :]).then_inc(s_out, 16)

        @block.scalar
        def _(scalar):
            # the HWDGE is one shared device (~560ns per DMA regardless of
            # issuing engine), so SP+ACT together get 5 gathers and the
            # independent SWDGE (gpsimd) takes the other 3.
            scalar.wait_ge(s_tp, 1)
            for cc in range(3, 5):
                gather(scalar, cc, s_gx)

        @block.gpsimd
        def _(gpsimd):
            gpsimd.memset(tpad[:, :], float(SENT)).then_inc(s_tpm, 1)
            gpsimd.memset(ones_k[:, :], 1.0)
            gpsimd.drain()
            gpsimd.affine_select(
                idn[:, :],
                ones_k[:, :],
                [[1, K]],
                mybir.AluOpType.is_equal,
                0.0,
                base=0,
                channel_multiplier=-1,
            ).then_inc(s_idn, 1)
            # pre-wake on the transpose sem so the Q7 is already spinning on
            # s_tp when it fires (cuts ~0.8us of gpsimd wake latency)
            gpsimd.wait_ge(s_pe, 1)
            gpsimd.wait_ge(s_tp, 1)
            for cc in range(5, NCORES):
                gather(gpsimd, cc, s_gx2)

        @block.tensor
        def _(tensor):
            tensor.wait_ge(s_idn, 1)
            tensor.wait_ge(s_t1, 1)
            tensor.transpose(tp_ps[:, :], t1[:, 0:S], idn[:, :]).then_inc(s_pe, 1)

        @block.vector
        def _(vector):
            vector.wait_ge(s_in, 16)
            vector.tensor_tensor(tmp1_w, xw_win, w_b1, add)
            vector.drain()
            vector.tensor_tensor(tmp1h_w, tmp1_l, tmp1_r, amax)
            vector.drain()
            vector.tensor_reduce(
                t1[:, :], tmp1h_w, axis=mybir.AxisListType.X, op=amax
            ).then_inc(s_t1, 1)
            vector.wait_ge(s_tpm, 1)
            vector.wait_ge(s_pe, 1)
            # tpad[r, 50 + p] = t1[p, r]
            vector.tensor_copy(tpad[0:S, PAD : PAD + K], tp_ps[:, :]).then_inc(
                s_tp, 1
            )
            vector.wait_ge(s_gx, 80)
            vector.wait_ge(s_gx2, 48)
            vector.tensor_tensor(tmp2_w, X_win, w_b2, add)
            vector.drain()
            vector.tensor_tensor(tmp2h_w, tmp2_l, tmp2_r, amax)
            vector.drain()
            vector.tensor_reduce(
                osb[:, :], tmp2h_w, axis=mybir.AxisListType.X, op=amax
            ).then_inc(s_o, 1)

    nc._in_init = False
    return nc


def _prep_in_maps(input, scale):
    inp = np.asarray(input, dtype=np.float32)
    s = np.float32(np.asarray(scale).reshape(()))

    z = (np.arange(K, dtype=np.float32) - np.float32(PAD)).astype(np.float32)
    wvec = (-(z * z) / (np.float32(4.0) * s)).astype(np.float16)
    w102 = np.full(JW, SENT, dtype=np.float16)
    w102[:K] = wvec

    rowpad = np.full((K, 224), SENT, dtype=np.float16)
    rowpad[:, PAD : PAD + K] = inp.astype(np.float16)

    in_maps = []
    for k in range(NCORES):
        xw = np.full((NP, XWC), SENT, dtype=np.float16)
        xw[:K, :WT] = rowpad[:, S * k : S * k + WT]
        xw[:, WT : WT + JW] = w102[None, :]
        in_maps.append({"xw": np.ascontiguousarray(xw)})
    return in_maps


def _unshard(results):
    out_full = np.empty((K, K), dtype=np.float32)
    for k, res in enumerate(results):
        o = np.asarray(res["out"]).astype(np.float32)  # [104, 14]
        blk = o.reshape(NCORES, S, WIN)[:, :, :S]  # [cc, r_loc, c_loc]
        blk = blk.transpose(1, 0, 2).reshape(S, NP)  # [r_loc, c]
        r0 = S * k
        nrows = min(S, K - r0)
        if nrows <= 0:
            continue
        out_full[r0 : r0 + nrows, :] = blk[:nrows, :K]
    return out_full


def kernel(input, scale):
    from concourse.bass_utils import run_bass_kernel_spmd

    if "nc" not in _CACHE:
        _CACHE["nc"] = _build_nc()
    nc = _CACHE["nc"]

    in_maps = _prep_in_maps(input, scale)
    res = run_bass_kernel_spmd(nc, in_maps, core_ids=list(range(NCORES)))
    return _unshard(res.results)


# revision 17
# speedup vs baseline: 1.1078x; 1.0007x over previous
"""Trainium2 Bass kernel for nn_Dilation2D (101x101 grayscale dilation with a
parabolic structuring element).

Math: out[r, c] = max_{i,j} padded[i + c, j + r] + h[i, j] with
h[i, j] = -(z_i^2 + z_j^2) / (4 s) separable into f(i) + g(j), so the 2D
max-plus convolution factors into two 1D sliding passes:

  stage 1:  t[p, r] = max_j rowpad[p, j + r] + w[j]     (slide along columns)
  stage 2:  out[r, c] = max_i tpad[i + c, r] + w[i]     (slide along rows)

with w[k] = -(k - 50)^2 / (4 s) and sentinel (-60000, fp16-safe) padding.

Sharding: output rows are split across the 8 cores (13 rows each). Each core
runs both stages restricted to its 13 output rows -- no cross-core
communication.

Layout (21.5us vs the 24us f32 baseline):
  * all compute in fp16 (tolerance is 2e-2; winning max candidates carry
    ~1e-3 relative error in fp16): tensor_tensor runs in the DVE 16-bit
    2x mode, and every DMA payload halves.
  * ONE input DMA: host packs the x window (128 cols) and the replicated
    w row (102 cols, sentinel-terminated) into one [104, 256] fp16 tensor
    (512B rows keep DMA descriptors out of the sub-512B penalty).
  * tensor_reduce gets no 16-bit speedup, so each reduce is preceded by a
    fold-in-half tensor_tensor(max) (j' vs j'+50, even 52-long runs) that
    runs at 2x and halves the 1x reduce work.
  * the stage-1->stage-2 replication (X[cc*13+r, u] = tpad[r, cc*13+u])
    must be 8 per-block DMAs: only dim0 of a DMA access pattern can cross
    SBUF partitions (a fused 3D-AP version is silently wrong on HW).
    The HWDGE is one shared device (~500ns/DMA serialized across SP+ACT),
    so the split is 3 on SP + 2 on ACT + 3 on gpsimd's independent SWDGE.
  * windows padded to 14 x 102 so access-pattern runs stay even-length
    (the DVE 16-bit 2x mode needs stride-1/2-byte runs of >= 2).
  * the remaining ~7us is the walrus BSP postamble (each engine serially
    resets its quarter of semaphores S[54..255], paced by the PE at
    ~115ns/instruction) -- independent of kernel content.
"""

import numpy as np

K = 101          # image/kernel size
PAD = 50
S = 13           # output rows per core
NCORES = 8
NP = NCORES * S  # 104
WT = 128         # x window columns held per partition
WIN = 14         # window positions computed per TT/RED (13 used + 1 pad)
JW = 102         # window length (101 used + 1 sentinel pad)
JH = 52          # folded half-window length (even, keeps 2x mode)
XWC = 256        # packed input row length: 128 x | 102 w | pad (512B rows)
TPC = 232        # tpad row length (needs >= 7*13 + 128 = 219)
SENT = np.float16(-60000.0)

_CACHE = {}


def _build_nc():
    import concourse.bass as bass
    import concourse.mybir as mybir

    f16 = mybir.dt.float16
    add = mybir.AluOpType.add
    amax = mybir.AluOpType.max

    class _FastBass(bass.Bass):
        # Bass.__init__ ends with an all-engine barrier that separates the
        # const-tensor memsets from user code; this kernel uses none of the
        # const tensors and every cross-engine handoff is semaphore-guarded,
        # so the barrier only adds startup latency. Skip it during
        # construction only.
        def all_engine_barrier(self):
            if getattr(self, "_in_init", True):
                return None
            return super().all_engine_barrier()

    nc = _FastBass(target_bir_lowering=False, debug=False, enable_asserts=False)

    xw_d = nc.dram_tensor("xw", [NP, XWC], f16, kind="ExternalInput")
    out_d = nc.dram_tensor("out", [NP, WIN], f16, kind="ExternalOutput")

    from contextlib import ExitStack

    with ExitStack() as stack:
        ec = stack.enter_context
        xw = ec(nc.sbuf_tensor("xw_s", [NP, XWC], f16))
        ones_k = ec(nc.sbuf_tensor("ones_k", [K, K], f16))
        idn = ec(nc.sbuf_tensor("idn", [K, K], f16))
        tmp1 = ec(nc.sbuf_tensor("tmp1", [K, WIN * JW], f16))
        t1 = ec(nc.sbuf_tensor("t1", [K, WIN], f16))
        tpad = ec(nc.sbuf_tensor("tpad", [S, TPC], f16))
        X = ec(nc.sbuf_tensor("X", [NP, WT], f16))
        tmp2 = ec(nc.sbuf_tensor("tmp2", [NP, WIN * JW], f16))
        tmp1h = ec(nc.sbuf_tensor("tmp1h", [K, WIN * JH], f16))
        tmp2h = ec(nc.sbuf_tensor("tmp2h", [NP, WIN * JH], f16))
        osb = ec(nc.sbuf_tensor("osb", [NP, WIN], f16))
        tp_ps = ec(nc.psum_tensor("tp_ps", [S, K], f16))
        s_in = ec(nc.semaphore("s_in"))
        s_idn = ec(nc.semaphore("s_idn"))
        s_tpm = ec(nc.semaphore("s_tpm"))
        s_t1 = ec(nc.semaphore("s_t1"))
        s_pe = ec(nc.semaphore("s_pe"))
        s_tp = ec(nc.semaphore("s_tp"))
        s_gx = ec(nc.semaphore("s_gx"))
        s_gx2 = ec(nc.semaphore("s_gx2"))
        s_o = ec(nc.semaphore("s_o"))
        s_out = ec(nc.semaphore("s_out"))
        block = ec(nc.Block())
        # stage 1: tmp1[p, rr, j] = xw[p, rr + j] + w[j]
        xw_win = bass.AP(xw, 0, [[XWC, K], [1, WIN], [1, JW]])
        w_b1 = bass.AP(xw, WT, [[XWC, K], [0, WIN], [1, JW]])
        tmp1_w = bass.AP(tmp1, 0, [[WIN * JW, K], [JW, WIN], [1, JW]])
        # stage 2: tmp2[P, c, i] = X[P, c + i] + w[i]
        X_win = bass.AP(X, 0, [[WT, NP], [1, WIN], [1, JW]])
        w_b2 = bass.AP(xw, WT, [[XWC, NP], [0, WIN], [1, JW]])
        tmp2_w = bass.AP(tmp2, 0, [[WIN * JW, NP], [JW, WIN], [1, JW]])
        # fold-in-half max: h[p, rr, j'] = max(tmp[p, rr, j'], tmp[p, rr, j'+50])
        # (j' in 0..51 covers 0..51 and 50..101; overlap is harmless for max,
        # and the 52-long even runs keep the DVE 16-bit 2x mode on)
        tmp1_l = bass.AP(tmp1, 0, [[WIN * JW, K], [JW, WIN], [1, JH]])
        tmp1_r = bass.AP(tmp1, JW - JH, [[WIN * JW, K], [JW, WIN], [1, JH]])
        tmp1h_w = bass.AP(tmp1h, 0, [[WIN * JH, K], [JH, WIN], [1, JH]])
        tmp2_l = bass.AP(tmp2, 0, [[WIN * JW, NP], [JW, WIN], [1, JH]])
        tmp2_r = bass.AP(tmp2, JW - JH, [[WIN * JW, NP], [JW, WIN], [1, JH]])
        tmp2h_w = bass.AP(tmp2h, 0, [[WIN * JH, NP], [JH, WIN], [1, JH]])

        def gather(eng, cc, sem):
            return eng.dma_start(
                X[cc * S : (cc + 1) * S, :],
                tpad[0:S, cc * S : cc * S + WT],
                single_packet=True,
            ).then_inc(sem, 16)

        @block.sync
        def _(sync):
            sync.dma_start(xw[:, :], xw_d[:, :]).then_inc(s_in, 16)
            sync.wait_ge(s_tp, 1)
            for cc in range(3):
                gather(sync, cc, s_gx)
            sync.wait_ge(s_o, 1)
            sync.dma_start(out_d[:, :], osb[:, :]).then_inc(s_out, 16)

        @block.scalar
        def _(scalar):
            # the HWDGE is one shared device (~560ns per DMA regardless of
            # issuing engine), so SP+ACT together get 5 gathers and the
            # independent SWDGE (gpsimd) takes the other 3.
            scalar.wait_ge(s_tp, 1)
            for cc in range(3, 5):
                gather(scalar, cc, s_gx)

        @block.gpsimd
        def _(gpsimd):
            gpsimd.memset(tpad[:, :], float(SENT)).then_inc(s_tpm, 1)
            gpsimd.memset(ones_k[:, :], 1.0)
            gpsimd.drain()
            gpsimd.affine_select(
                idn[:, :],
                ones_k[:, :],
                [[1, K]],
                mybir.AluOpType.is_equal,
                0.0,
                base=0,
                channel_multiplier=-1,
            ).then_inc(s_idn, 1)
            gpsimd.wait_ge(s_tp, 1)
            for cc in range(5, NCORES):
                gather(gpsimd, cc, s_gx2)

        @block.tensor
        def _(tensor):
            tensor.wait_ge(s_idn, 1)
            tensor.wait_ge(s_t1, 1)
            tensor.transpose(tp_ps[:, :], t1[:, 0:S], idn[:, :]).then_inc(s_pe, 1)

        @block.vector
        def _(vector):
            vector.wait_ge(s_in, 16)
            vector.tensor_tensor(tmp1_w, xw_win, w_b1, add)
            vector.drain()
            vector.tensor_tensor(tmp1h_w, tmp1_l, tmp1_r, amax)
            vector.drain()
            vector.tensor_reduce(
                t1[:, :], tmp1h_w, axis=mybir.AxisListType.X, op=amax
            ).then_inc(s_t1, 1)
            vector.wait_ge(s_tpm, 1)
            vector.wait_ge(s_pe, 1)
            # tpad[r, 50 + p] = t1[p, r]
            vector.tensor_copy(tpad[0:S, PAD : PAD + K], tp_ps[:, :]).then_inc(
                s_tp, 1
            )
            vector.wait_ge(s_gx, 80)
            vector.wait_ge(s_gx2, 48)
            vector.tensor_tensor(tmp2_w, X_win, w_b2, add)
            vector.drain()
            vector.tensor_tensor(tmp2h_w, tmp2_l, tmp2_r, amax)
            vector.drain()
            vector.tensor_reduce(
                osb[:, :], tmp2h_w, axis=mybir.AxisListType.X, op=amax
            ).then_inc(s_o, 1)

    nc._in_init = False
    return nc


def _prep_in_maps(input, scale):
    inp = np.asarray(input, dtype=np.float32)
    s = np.float32(np.asarray(scale).reshape(()))

    z = (np.arange(K, dtype=np.float32) - np.float32(PAD)).astype(np.float32)
    wvec = (-(z * z) / (np.float32(4.0) * s)).astype(np.float16)
    w102 = np.full(JW, SENT, dtype=np.float16)
    w102[:K] = wvec

    rowpad = np.full((K, 224), SENT, dtype=np.float16)
    rowpad[:, PAD : PAD + K] = inp.astype(np.float16)

    in_maps = []
    for k in range(NCORES):
        xw = np.full((NP, XWC), SENT, dtype=np.float16)
        xw[:K, :WT] = rowpad[:, S * k : S * k + WT]
        xw[:, WT : WT + JW] = w102[None, :]
        in_maps.append({"xw": np.ascontiguousarray(xw)})
    return in_maps


def _unshard(results):
    out_full = np.empty((K, K), dtype=np.float32)
    for k, res in enumerate(results):
        o = np.asarray(res["out"]).astype(np.float32)  # [104, 14]
        blk = o.reshape(NCORES, S, WIN)[:, :, :S]  # [cc, r_loc, c_loc]
        blk = blk.transpose(1, 0, 2).reshape(S, NP)  # [r_loc, c]
        r0 = S * k
        nrows = min(S, K - r0)
        if nrows <= 0:
            continue
        out_full[r0 : r0 + nrows, :] = blk[:nrows, :K]
    return out_full


def kernel(input, scale):
    from concourse.bass_utils import run_bass_kernel_spmd

    if "nc" not in _CACHE:
        _CACHE["nc"] = _build_nc()
    nc = _CACHE["nc"]

    in_maps = _prep_in_maps(input, scale)
    res = run_bass_kernel_spmd(nc, in_maps, core_ids=list(range(NCORES)))
    return _unshard(res.results)


# revision 18
# speedup vs baseline: 1.1118x; 1.0036x over previous
"""Trainium2 Bass kernel for nn_Dilation2D (101x101 grayscale dilation with a
parabolic structuring element).

Math: out[r, c] = max_{i,j} padded[i + c, j + r] + h[i, j] with
h[i, j] = -(z_i^2 + z_j^2) / (4 s) separable into f(i) + g(j), so the 2D
max-plus convolution factors into two 1D sliding passes:

  stage 1:  t[p, r] = max_j rowpad[p, j + r] + w[j]     (slide along columns)
  stage 2:  out[r, c] = max_i tpad[i + c, r] + w[i]     (slide along rows)

with w[k] = -(k - 50)^2 / (4 s) and sentinel (-60000, fp16-safe) padding.

Sharding: output rows are split across the 8 cores (13 rows each). Each core
runs both stages restricted to its 13 output rows -- no cross-core
communication.

Layout (21.5us vs the 24us f32 baseline):
  * all compute in fp16 (tolerance is 2e-2; winning max candidates carry
    ~1e-3 relative error in fp16): tensor_tensor runs in the DVE 16-bit
    2x mode, and every DMA payload halves.
  * ONE input DMA: host packs the x window (128 cols) and the replicated
    w row (102 cols, sentinel-terminated) into one [104, 256] fp16 tensor
    (512B rows keep DMA descriptors out of the sub-512B penalty).
  * tensor_reduce gets no 16-bit speedup, so each reduce is preceded by a
    fold-in-half tensor_tensor(max) (j' vs j'+50, even 52-long runs) that
    runs at 2x and halves the 1x reduce work.
  * the stage-1->stage-2 replication (X[cc*13+r, u] = tpad[r, cc*13+u])
    must be 8 per-block DMAs: only dim0 of a DMA access pattern can cross
    SBUF partitions (a fused 3D-AP version is silently wrong on HW).
    The HWDGE is one shared device (~500ns/DMA serialized across SP+ACT),
    so the split is 3 on SP + 2 on ACT + 3 on gpsimd's independent SWDGE.
  * windows padded to 14 x 102 so access-pattern runs stay even-length
    (the DVE 16-bit 2x mode needs stride-1/2-byte runs of >= 2).
  * the remaining ~7us is the walrus BSP postamble (each engine serially
    resets its quarter of semaphores S[54..255], paced by the PE at
    ~115ns/instruction) -- independent of kernel content.
"""

import numpy as np

K = 101          # image/kernel size
PAD = 50
S = 13           # output rows per core
NCORES = 8
NP = NCORES * S  # 104
WT = 128         # x window columns held per partition
WIN = 14         # window positions computed per TT/RED (13 used + 1 pad)
JW = 102         # window length (101 used + 1 sentinel pad)
JH = 52          # folded half-window length (even, keeps 2x mode)
XWC = 256        # packed input row length: 128 x | 102 w | pad (512B rows)
TPC = 232        # tpad row length (needs >= 7*13 + 128 = 219)
SENT = np.float16(-60000.0)

_CACHE = {}


def _build_nc():
    import concourse.bass as bass
    import concourse.mybir as mybir

    f16 = mybir.dt.float16
    add = mybir.AluOpType.add
    amax = mybir.AluOpType.max

    class _FastBass(bass.Bass):
        # Bass.__init__ ends with an all-engine barrier that separates the
        # const-tensor memsets from user code; this kernel uses none of the
        # const tensors and every cross-engine handoff is semaphore-guarded,
        # so the barrier only adds startup latency. Skip it during
        # construction only.
        def all_engine_barrier(self):
            if getattr(self, "_in_init", True):
                return None
            return super().all_engine_barrier()

    nc = _FastBass(target_bir_lowering=False, debug=False, enable_asserts=False)

    xw_d = nc.dram_tensor("xw", [NP, XWC], f16, kind="ExternalInput")
    out_d = nc.dram_tensor("out", [NP, WIN], f16, kind="ExternalOutput")

    from contextlib import ExitStack

    with ExitStack() as stack:
        ec = stack.enter_context
        xw = ec(nc.sbuf_tensor("xw_s", [NP, XWC], f16))
        ones_k = ec(nc.sbuf_tensor("ones_k", [K, K], f16))
        idn = ec(nc.sbuf_tensor("idn", [K, K], f16))
        tmp1 = ec(nc.sbuf_tensor("tmp1", [K, WIN * JW], f16))
        t1 = ec(nc.sbuf_tensor("t1", [K, WIN], f16))
        tpad = ec(nc.sbuf_tensor("tpad", [S, TPC], f16))
        X = ec(nc.sbuf_tensor("X", [NP, WT], f16))
        tmp2 = ec(nc.sbuf_tensor("tmp2", [NP, WIN * JW], f16))
        tmp1h = ec(nc.sbuf_tensor("tmp1h", [K, WIN * JH], f16))
        tmp2h = ec(nc.sbuf_tensor("tmp2h", [NP, WIN * JH], f16))
        osb = ec(nc.sbuf_tensor("osb", [NP, WIN], f16))
        tp_ps = ec(nc.psum_tensor("tp_ps", [S, K], f16))
        s_in = ec(nc.semaphore("s_in"))
        s_idn = ec(nc.semaphore("s_idn"))
        s_tpm = ec(nc.semaphore("s_tpm"))
        s_t1 = ec(nc.semaphore("s_t1"))
        s_pe = ec(nc.semaphore("s_pe"))
        s_tp = ec(nc.semaphore("s_tp"))
        s_gx = ec(nc.semaphore("s_gx"))
        s_gx2 = ec(nc.semaphore("s_gx2"))
        s_o = ec(nc.semaphore("s_o"))
        s_out = ec(nc.semaphore("s_out"))
        block = ec(nc.Block())
        # stage 1: tmp1[p, rr, j] = xw[p, rr + j] + w[j]
        xw_win = bass.AP(xw, 0, [[XWC, K], [1, WIN], [1, JW]])
        w_b1 = bass.AP(xw, WT, [[XWC, K], [0, WIN], [1, JW]])
        tmp1_w = bass.AP(tmp1, 0, [[WIN * JW, K], [JW, WIN], [1, JW]])
        # stage 2: tmp2[P, c, i] = X[P, c + i] + w[i]
        X_win = bass.AP(X, 0, [[WT, NP], [1, WIN], [1, JW]])
        w_b2 = bass.AP(xw, WT, [[XWC, NP], [0, WIN], [1, JW]])
        tmp2_w = bass.AP(tmp2, 0, [[WIN * JW, NP], [JW, WIN], [1, JW]])
        # fold-in-half max: h[p, rr, j'] = max(tmp[p, rr, j'], tmp[p, rr, j'+50])
        # (j' in 0..51 covers 0..51 and 50..101; overlap is harmless for max,
        # and the 52-long even runs keep the DVE 16-bit 2x mode on)
        tmp1_l = bass.AP(tmp1, 0, [[WIN * JW, K], [JW, WIN], [1, JH]])
        tmp1_r = bass.AP(tmp1, JW - JH, [[WIN * JW, K], [JW, WIN], [1, JH]])
        tmp1h_w = bass.AP(tmp1h, 0, [[WIN * JH, K], [JH, WIN], [1, JH]])
        tmp2_l = bass.AP(tmp2, 0, [[WIN * JW, NP], [JW, WIN], [1, JH]])
        tmp2_r = bass.AP(tmp2, JW - JH, [[WIN * JW, NP], [JW, WIN], [1, JH]])
        tmp2h_w = bass.AP(tmp2h, 0, [[WIN * JH, NP], [JH, WIN], [1, JH]])

        def gather(eng, cc, sem):
            return eng.dma_start(
                X[cc * S : (cc + 1) * S, :],
                tpad[0:S, cc * S : cc * S + WT],
                single_packet=False,
            ).then_inc(sem, 16)

        @block.sync
        def _(sync):
            sync.dma_start(xw[:, :], xw_d[:, :]).then_inc(s_in, 16)
            sync.wait_ge(s_tp, 1)
            for cc in range(3):
                gather(sync, cc, s_gx)
            sync.wait_ge(s_o, 1)
            sync.dma_start(out_d[:, :], osb[:, :]).then_inc(s_out, 16)

        @block.scalar
        def _(scalar):
            # the HWDGE is one shared device (~560ns per DMA regardless of
            # issuing engine), so SP+ACT together get 5 gathers and the
            # independent SWDGE (gpsimd) takes the other 3.
            scalar.wait_ge(s_tp, 1)
            for cc in range(3, 5):
                gather(scalar, cc, s_gx)

        @block.gpsimd
        def _(gpsimd):
            gpsimd.memset(tpad[:, :], float(SENT)).then_inc(s_tpm, 1)
            gpsimd.memset(ones_k[:, :], 1.0)
            gpsimd.drain()
            gpsimd.affine_select(
                idn[:, :],
                ones_k[:, :],
                [[1, K]],
                mybir.AluOpType.is_equal,
                0.0,
                base=0,
                channel_multiplier=-1,
            ).then_inc(s_idn, 1)
            gpsimd.wait_ge(s_tp, 1)
            for cc in range(5, NCORES):
                gather(gpsimd, cc, s_gx2)

        @block.tensor
        def _(tensor):
            tensor.wait_ge(s_idn, 1)
            tensor.wait_ge(s_t1, 1)
            tensor.transpose(tp_ps[:, :], t1[:, 0:S], idn[:, :]).then_inc(s_pe, 1)

        @block.vector
        def _(vector):
            vector.wait_ge(s_in, 16)
            vector.tensor_tensor(tmp1_w, xw_win, w_b1, add)
            vector.drain()
            vector.tensor_tensor(tmp1h_w, tmp1_l, tmp1_r, amax)
            vector.drain()
            vector.tensor_reduce(
                t1[:, :], tmp1h_w, axis=mybir.AxisListType.X, op=amax
            ).then_inc(s_t1, 1)
            vector.wait_ge(s_tpm, 1)
            vector.wait_ge(s_pe, 1)
            # tpad[r, 50 + p] = t1[p, r]
            vector.tensor_copy(tpad[0:S, PAD : PAD + K], tp_ps[:, :]).then_inc(
                s_tp, 1
            )
            vector.wait_ge(s_gx, 80)
            vector.wait_ge(s_gx2, 48)
            vector.tensor_tensor(tmp2_w, X_win, w_b2, add)
            vector.drain()
            vector.tensor_tensor(tmp2h_w, tmp2_l, tmp2_r, amax)
            vector.drain()
            vector.tensor_reduce(
                osb[:, :], tmp2h_w, axis=mybir.AxisListType.X, op=amax
            ).then_inc(s_o, 1)

    nc._in_init = False
    return nc


def _prep_in_maps(input, scale):
    inp = np.asarray(input, dtype=np.float32)
    s = np.float32(np.asarray(scale).reshape(()))

    z = (np.arange(K, dtype=np.float32) - np.float32(PAD)).astype(np.float32)
    wvec = (-(z * z) / (np.float32(4.0) * s)).astype(np.float16)
    w102 = np.full(JW, SENT, dtype=np.float16)
    w102[:K] = wvec

    rowpad = np.full((K, 224), SENT, dtype=np.float16)
    rowpad[:, PAD : PAD + K] = inp.astype(np.float16)

    in_maps = []
    for k in range(NCORES):
        xw = np.full((NP, XWC), SENT, dtype=np.float16)
        xw[:K, :WT] = rowpad[:, S * k : S * k + WT]
        xw[:, WT : WT + JW] = w102[None, :]
        in_maps.append({"xw": np.ascontiguousarray(xw)})
    return in_maps


def _unshard(results):
    out_full = np.empty((K, K), dtype=np.float32)
    for k, res in enumerate(results):
        o = np.asarray(res["out"]).astype(np.float32)  # [104, 14]
        blk = o.reshape(NCORES, S, WIN)[:, :, :S]  # [cc, r_loc, c_loc]
        blk = blk.transpose(1, 0, 2).reshape(S, NP)  # [r_loc, c]
        r0 = S * k
        nrows = min(S, K - r0)
        if nrows <= 0:
            continue
        out_full[r0 : r0 + nrows, :] = blk[:nrows, :K]
    return out_full


def kernel(input, scale):
    from concourse.bass_utils import run_bass_kernel_spmd

    if "nc" not in _CACHE:
        _CACHE["nc"] = _build_nc()
    nc = _CACHE["nc"]

    in_maps = _prep_in_maps(input, scale)
    res = run_bass_kernel_spmd(nc, in_maps, core_ids=list(range(NCORES)))
    return _unshard(res.results)
